# revision 1
# baseline (speedup 1.0000x reference)
"""Trainium2 Bass kernel for nn_Attention_77824807403911 (sparse_attention).

Math (per batch element, no softmax => associativity):
    q = x @ Wq^T + bq ; v = x @ Wv^T + bv          [1024, 256]
    rq = rope(q) ; rv = rope(v)
    per head h (16 heads, hd=16):  att_h = rq_h @ (rq_h^T @ rv_h) / 4
    out = att @ Wo^T + bo

Instead of the 1024x1024 score matrix we compute the 16x16 Gram per head
(64x fewer flops), realized as a full 256x256 Gram masked to the
block-diagonal, folded with Wo into a single per-batch [256,256] weight:
    F[e,f]  = sum_s rv[s,e] rq[s,f]       (Gram, transposed blocks)
    BDT     = F .* blockmask
    W2[f,o] = sum_e BDT[e,f] * Wo[o,e] / 4
    outT    = W2^T @ rqT + bo             ([256, 1024])

Sharding: data-parallel over batch, 1 element per core, no collectives.

v6 deltas over the original schedule:
- input DMA ring order puts weights + x first, trig halves last (the
  rings drain roughly FIFO; rope needs trig only ~3us after the first
  projection matmul);
- x split into 512-col halves so the second projection chunk isn't
  gated on the whole 256KB row block;
- output stored bf16 (host upcasts) halving output traffic, with the
  four output DMAs alternating across both HWDGE rings;
- eviction order ac-major so full-width rope starts after 2 evicts.
"""

import numpy as np
import ml_dtypes

import concourse.bass as bass
import concourse.bacc as bacc
import concourse.tile as tile
from concourse import mybir
from concourse.bass_utils import run_bass_kernel_spmd

B, S, D, H, HD = 8, 1024, 256, 16, 16
N_CORES = 8
BF16 = mybir.dt.bfloat16
F32 = mybir.dt.float32

PERM = np.concatenate(
    [np.arange(0, 128, 2), np.arange(128, 256, 2),
     np.arange(1, 128, 2), np.arange(129, 256, 2)]
)


def _host_tables():
    p = np.arange(128)
    theta = np.where(p < 64, 1.0, 1e-4)
    s = np.arange(S, dtype=np.float64) + 1.0
    ang = theta[:, None] * s[None, :]
    sin = np.sin(ang).astype(ml_dtypes.bfloat16)
    cos = np.cos(ang).astype(ml_dtypes.bfloat16)
    a = np.arange(256)
    headp = (a % 128) // 8
    mask = (headp[:, None] == headp[None, :]).astype(ml_dtypes.bfloat16)
    return sin, cos, mask


def build_kernel():
    nc = bacc.Bacc()
    xT = nc.declare_dram_parameter("xT", [D, S], BF16, isOutput=False)
    # wbig columns: [wq(256) | wv(256) | bias(3) | pad(1) | trig(1024: sin on rows
    # 0:128, cos on rows 128:256) | wo(256) | mask(256)]
    wbig = nc.declare_dram_parameter("wbig", [D, 4 * D + 4 + S], BF16,
                                     isOutput=False)
    outT = nc.declare_dram_parameter("outT", [D, S], BF16, isOutput=True)

    with tile.TileContext(nc) as tc:
        _body(tc, xT, wbig, outT)
    nc.compile()
    return nc


def _body(tc, xT, wbig, outT):
    nc = tc.nc
    NS = 2          # s chunks of 512 for matmul streaming
    SC = S // NS    # 512

    with (
        tc.tile_pool(name="const", bufs=1) as cpool,
        tc.tile_pool(name="acts", bufs=1) as apool,
        tc.tile_pool(name="psum", bufs=4, space="PSUM") as pp,
        tc.tile_pool(name="outp", bufs=4) as opool,
    ):
        # PE warm-up: garbage matmuls release the HAM clock gate while the
        # real inputs stream in. scratch is memset early in the preamble.
        scratch = cpool.tile([128, 512], BF16, tag="scratch", name="scratch")
        nc.gpsimd.memset(scratch[:], 0.25)
        warm_ps = pp.tile([128, 512], F32, tag="warm", bufs=1, name="warm_ps")
        for wi in range(8):
            nc.tensor.matmul(warm_ps[:], scratch[:, 0:128], scratch[:],
                             start=True, stop=True, skip_group_check=True)

        # ---- input DMAs: 4 per ring; heads are [wq|wv|bias] then x halves ----
        WB = 4 * D + 4 + S      # 2052 packed weight columns
        A_END = 2 * D + 4       # wq|wv|bias|pad piece (trig 4B-aligned)
        xT_sb, w_sb = [], []
        for cc in range(2):
            xT_sb.append(cpool.tile([128, S], BF16, tag=f"xT{cc}", name=f"xT{cc}"))
            w_sb.append(cpool.tile([128, WB], BF16, tag=f"wbig{cc}",
                                   name=f"wbig{cc}"))
        nc.sync.dma_start(w_sb[0][:, 0:A_END], wbig[0:128, 0:A_END])
        nc.scalar.dma_start(w_sb[1][:, 0:A_END], wbig[128:256, 0:A_END])
        nc.sync.dma_start(xT_sb[0][:, 0:SC], xT[0:128, 0:SC])
        nc.scalar.dma_start(xT_sb[1][:, 0:SC], xT[128:256, 0:SC])
        nc.sync.dma_start(xT_sb[0][:, SC:S], xT[0:128, SC:S])
        nc.scalar.dma_start(xT_sb[1][:, SC:S], xT[128:256, SC:S])
        nc.sync.dma_start(w_sb[0][:, A_END:WB], wbig[0:128, A_END:WB])
        nc.scalar.dma_start(w_sb[1][:, A_END:WB], wbig[128:256, A_END:WB])

        sin_sb = w_sb[0][:, A_END:A_END + S]
        cos_sb = w_sb[1][:, A_END:A_END + S]
        _WBASE = {0: 0, 1: D, 2: A_END + S, 3: A_END + S + D}  # wq, wv, wo, mask

        def wslice(idx, cc, col0, ncol):
            base = _WBASE[idx]
            return w_sb[cc][:, base + col0: base + col0 + ncol]

        def bias_ap(idx, cc):
            return w_sb[cc][:, 2 * D + idx: 2 * D + idx + 1]

        def act2(tag, width=S, dtype=BF16):
            return [apool.tile([128, width], dtype, tag=f"{tag}{cc}",
                               name=f"{tag}{cc}") for cc in range(2)]

        # f32 copy of the output bias for the DVE-side final evicts
        bo32 = [cpool.tile([128, 1], F32, tag=f"bo32_{oc}", name=f"bo32_{oc}")
                for oc in range(2)]
        for oc in range(2):
            nc.scalar.activation(bo32[oc][:], bias_ap(2, oc),
                                 mybir.ActivationFunctionType.Copy)

        qT = act2("qT")
        vT = act2("vT")
        rqT = act2("rqT")
        rvT = act2("rvT")
        # natural-layout tiles padded to 272-col st-blocks: breaks the 4KB
        # power-of-2 row-stride SBUF bank pattern that slows gram LDWEIGHTS
        DP = D + 16
        rq_nat = apool.tile([128, 8 * DP], BF16, tag="rq_nat")
        rv_nat = apool.tile([128, 8 * DP], BF16, tag="rv_nat")

        # ---- projections: tT[a, s] = sum_d w[d, a] x[d, s] ----
        ps_map = {}

        def proj_chunk(widx, sc):
            for ac in range(2):
                ps = pp.tile([128, SC], F32, tag="mm", bufs=5,
                             name=f"proj_ps{widx}{ac}{sc}")
                for dc in range(2):
                    nc.tensor.matmul(
                        ps[:],
                        wslice(widx, dc, ac * 128, 128),
                        xT_sb[dc][:, sc * SC:(sc + 1) * SC],
                        start=(dc == 0), stop=(dc == 1),
                    )
                ps_map[(widx, ac, sc)] = ps

        # PE order: q-s0, v-s0 (x second halves land later), q-s1, v-s1
        proj_chunk(0, 0)
        proj_chunk(1, 0)
        proj_chunk(0, 1)
        proj_chunk(1, 1)

        def evict(widx, dstT, bidx, ac, sc):
            nc.scalar.activation(
                dstT[ac][:, sc * SC:(sc + 1) * SC],
                ps_map[(widx, ac, sc)][:],
                mybir.ActivationFunctionType.Identity,
                bias=bias_ap(bidx, ac),
            )

        evict(0, qT, 0, 0, 0)
        evict(0, qT, 0, 1, 0)
        evict(1, vT, 1, 0, 0)
        evict(1, vT, 1, 1, 0)
        evict(0, qT, 0, 0, 1)
        evict(0, qT, 0, 1, 1)
        evict(1, vT, 1, 0, 1)
        evict(1, vT, 1, 1, 1)

        # ---- rope: q per s-chunk (starts after 2 evicts), v full-width ----
        def rope_chunk(srcT, dstT, sc, tmp_tag):
            sl = slice(sc * SC, (sc + 1) * SC)
            E, O = srcT[0][:, sl], srcT[1][:, sl]
            ssl = sin_sb[:, sl]
            csl = cos_sb[:, sl]
            m1 = opool.tile([128, SC], BF16, tag="m1", bufs=2)
            m2 = opool.tile([128, SC], BF16, tag="m2", bufs=2)
            m3 = opool.tile([128, SC], BF16, tag="m3", bufs=2)
            m4 = opool.tile([128, SC], BF16, tag="m4", bufs=2)
            nc.vector.tensor_tensor(m1[:], E, ssl, mybir.AluOpType.mult)
            nc.vector.tensor_tensor(m3[:], E, csl, mybir.AluOpType.mult)
            nc.vector.tensor_tensor(m2[:], O, csl, mybir.AluOpType.mult)
            nc.vector.tensor_tensor(m4[:], O, ssl, mybir.AluOpType.mult)
            nc.vector.tensor_tensor(dstT[0][:, sl], m1[:], m2[:],
                                    mybir.AluOpType.subtract)
            nc.vector.tensor_tensor(dstT[1][:, sl], m3[:], m4[:],
                                    mybir.AluOpType.add)

        def rope_full(srcT, dstT, tmp_tag):
            E, O = srcT[0][:], srcT[1][:]
            t1 = opool.tile([128, S], BF16, tag=tmp_tag + "1", name=tmp_tag + "1")
            t2 = opool.tile([128, S], BF16, tag=tmp_tag + "2", name=tmp_tag + "2")
            nc.vector.tensor_tensor(t1[:], E, sin_sb, mybir.AluOpType.mult)
            nc.vector.tensor_tensor(t2[:], O, cos_sb, mybir.AluOpType.mult)
            nc.vector.tensor_tensor(dstT[0][:], t1[:], t2[:], mybir.AluOpType.subtract)
            t3 = opool.tile([128, S], BF16, tag=tmp_tag + "3", name=tmp_tag + "3")
            t4 = opool.tile([128, S], BF16, tag=tmp_tag + "4", name=tmp_tag + "4")
            nc.vector.tensor_tensor(t3[:], E, cos_sb, mybir.AluOpType.mult)
            nc.vector.tensor_tensor(t4[:], O, sin_sb, mybir.AluOpType.mult)
            nc.vector.tensor_tensor(dstT[1][:], t3[:], t4[:], mybir.AluOpType.add)

        rq_nat3 = rq_nat[:].rearrange("p (st c) -> p st c", c=DP)
        rv_nat3 = rv_nat[:].rearrange("p (st c) -> p st c", c=DP)

        rope_chunk(qT, rqT, 0, "rq0")
        for cc in range(2):     # t(q,s0): scalar busy with evicts -> sync
            nc.sync.dma_start(rq_nat3[:, 0:4, cc * 128:(cc + 1) * 128],
                              rqT[cc][:, 0:SC], transpose=True)
        rope_chunk(qT, rqT, 1, "rq1")
        nc.sync.dma_start(rq_nat3[:, 4:8, 0:128], rqT[0][:, SC:S], transpose=True)
        nc.scalar.dma_start(rq_nat3[:, 4:8, 128:256], rqT[1][:, SC:S],
                            transpose=True)
        rope_full(vT, rvT, "rv_tmp")
        nc.sync.dma_start(rv_nat3[:, :, 0:128], rvT[0][:], transpose=True)
        nc.scalar.dma_start(rv_nat3[:, :, 128:256], rvT[1][:], transpose=True)

        # keep the PE clock-gate open across the rope gap
        for wi in range(8):
            srcv = vT[wi % 2]
            nc.tensor.matmul(warm_ps[:], srcv[:, 0:128], srcv[:, 0:512],
                             start=True, stop=True, skip_group_check=True)

        # ---- Gram: Hm[e, f] = sum_s rv[s, e] rq[s, f]; mask -> BDT ----
        bdt = act2("bdt", width=D)
        for ec in range(2):
            ps = pp.tile([128, D], F32, tag="sm", bufs=2, name=f"gram_ps{ec}")
            for st in range(8):
                nc.tensor.matmul(
                    ps[:],
                    rv_nat[:, st * DP + ec * 128: st * DP + (ec + 1) * 128],
                    rq_nat[:, st * DP: st * DP + D],
                    start=(st == 0), stop=(st == 7),
                )
            nc.vector.tensor_tensor(
                bdt[ec][:], ps[:], wslice(3, ec, 0, D), mybir.AluOpType.mult)

        # ---- W2[f, o] = sum_e BDT[e, f] wot[e, o] (scaled 1/4 at evict) ----
        w2 = act2("w2", width=D)
        for fc in range(2):
            ps = pp.tile([128, D], F32, tag="sm", bufs=2, name=f"w2_ps{fc}")
            for ec in range(2):
                nc.tensor.matmul(
                    ps[:],
                    bdt[ec][:, fc * 128:(fc + 1) * 128],
                    wslice(2, ec, 0, D),
                    start=(ec == 0), stop=(ec == 1),
                )
            if fc == 0:
                nc.scalar.activation(
                    w2[fc][:], ps[:],
                    mybir.ActivationFunctionType.Copy, scale=0.25)
            else:
                nc.vector.tensor_scalar_mul(w2[fc][:], ps[:], 0.25)

        # ---- final: outT[o, s] = sum_f W2[f, o] rqT[f, s] + bo ----
        for oc in range(2):
            for sc in range(NS):
                ps = pp.tile([128, SC], F32, tag="mm", bufs=5,
                             name=f"fin_ps{oc}{sc}")
                for fc in range(2):
                    nc.tensor.matmul(
                        ps[:],
                        w2[fc][:, oc * 128:(oc + 1) * 128],
                        rqT[fc][:, sc * SC:(sc + 1) * SC],
                        start=(fc == 0), stop=(fc == 1),
                    )
                ot = opool.tile([128, SC], BF16, tag="out_sb", name=f"out_sb{oc}{sc}")
                if sc == 0:
                    nc.scalar.activation(
                        ot[:], ps[:],
                        mybir.ActivationFunctionType.Identity,
                        bias=bias_ap(2, oc),
                    )
                else:
                    nc.vector.tensor_scalar(ot[:], ps[:], bo32[oc][:], None,
                                            mybir.AluOpType.add)
                eng = nc.scalar if (oc + sc) % 2 == 0 else nc.sync
                eng.dma_start(
                    outT[oc * 128:(oc + 1) * 128, sc * SC:(sc + 1) * SC], ot[:])


_NC_CACHE = None


def _get_nc():
    global _NC_CACHE
    if _NC_CACHE is None:
        _NC_CACHE = build_kernel()
    return _NC_CACHE


def make_in_maps(x, wq_w, wq_b, wv_w, wv_b, wo_w, wo_b):
    sin, cos, mask = _host_tables()
    wq_p = np.ascontiguousarray(wq_w[PERM].T).astype(ml_dtypes.bfloat16)
    wv_p = np.ascontiguousarray(wv_w[PERM].T).astype(ml_dtypes.bfloat16)
    wo_p = np.ascontiguousarray(wo_w[:, PERM].T).astype(ml_dtypes.bfloat16)
    bias3 = np.stack([wq_b[PERM], wv_b[PERM], wo_b], axis=1).astype(ml_dtypes.bfloat16)
    # trig block: sin on rows 0:128, cos on rows 128:256
    trig = np.concatenate([sin, cos], axis=0).astype(ml_dtypes.bfloat16)
    wbig = np.ascontiguousarray(
        np.concatenate([wq_p, wv_p, bias3,
                        np.zeros((256, 1), dtype=ml_dtypes.bfloat16),
                        trig, wo_p, mask], axis=1))
    in_maps = []
    for b in range(B):
        in_maps.append({
            "xT": np.ascontiguousarray(x[b].T).astype(ml_dtypes.bfloat16),
            "wbig": wbig,
        })
    return in_maps


TRACE = False
RUN_KWARGS = {}
LAST_RESULT = None


def kernel(x, wq_w, wq_b, wk_w, wk_b, wv_w, wv_b, wo_w, wo_b):
    global LAST_RESULT
    x = np.asarray(x, dtype=np.float32)
    in_maps = make_in_maps(x, np.asarray(wq_w, np.float32), np.asarray(wq_b, np.float32),
                           np.asarray(wv_w, np.float32), np.asarray(wv_b, np.float32),
                           np.asarray(wo_w, np.float32), np.asarray(wo_b, np.float32))
    nc = _get_nc()
    res = run_bass_kernel_spmd(nc, in_maps, core_ids=list(range(N_CORES)),
                               trace=TRACE, **RUN_KWARGS)
    LAST_RESULT = res
    outs = [np.ascontiguousarray(res.results[b]["outT"].T) for b in range(B)]
    return np.stack(outs).astype(np.float32)

